# revision 46
# baseline (speedup 1.0000x reference)
"""GAU (gated attention unit) Trainium2 Bass kernel, 8-core SPMD.

Problem: B=4, T=2048, D=1024, DF=2048, S=128, fp32 in/out.
  u = silu(x@Wu+bu); v = silu(x@Wv+bv); z = silu(x@Wqk+bqk)
  q = (z*g0+b0)/sqrt(S); k = z*g1+b1
  scores = (q+u_qk) @ k^T, key-masked by length[b]; attn = softmax
  out = u * (attn@v); y = out@Wo + bo

Sharding: core c -> batch b=c//2, token half h=c%2 (1024 tokens).
Each core computes u/q for its own token half (its query half) AND
v/k only for its own token half; the partner's k/v half arrives via a
pair-wise "sum exchange":

  Each core writes (k, v) * m_s into shard s of a ReduceScatter(add)
  input, where the host supplies m = (0,1) on even cores and (1,0) on
  odd cores.  Rank r's RS output is shard r = 0 + partner's data, so
  every core receives exactly its partner's (k, v) -- SPMD-symmetric
  (no core-dependent addressing), numerically exact, and the
  collective is priced/sized on the RECEIVED bytes only.

Three RSs pipeline under the projections: k (tiny, early), then
v[:, :1024], then v[:, 1024:], each launched as soon as its slice is
staged.  This removes the duplicated full-T v/z compute of the pure
data-parallel layout (~46 us of PE time per core).

Keys are in LOCAL-ROTATED order (own half first), as in the mask the
host builds; attention is key-permutation invariant.

Layout strategy (everything stays transposed so no on-device
transposes are needed; host pre-transposes x and pre-packs weights):
  xtT  [d, tq]  : rhs/lhsT for all projections (contract d on partitions)
  z    [s, tq]  = (Wqk lhsT) @ (xtT rhs)         -> qT [s,tq], k_own
  v    [tk, f]  = (xtT lhsT) @ (Wv rhs)          own j tiles 0..7
  uT   [f, tq]  = (Wu lhsT) @ (xtT rhs)
  pT   [tk,tq]  = exp((kT lhsT)@(qT rhs) + mask) softmax numerator
  den  [tq, 1]  = (pT lhsT) @ (ones rhs)         per-query denominator
  oT   [f, tq]  = (v lhsT) @ (pT rhs), gated *uT
  y    [tq, d]  = (oT lhsT) @ (Wo rhs), *1/den, +bo

All matmuls in bf16 with fp32 PSUM accumulation. Softmax skips the
row-max subtraction: pre-softmax logits for this operator are
|s| <~ 2 (q is scaled by gamma*0.02-ish weights and 1/sqrt(S)), so
exp() cannot overflow; masked keys get a -1e30 bias -> exp==0 exactly.

DMAs are spread across the sync/scalar/gpsimd queues (a DMA's cost is
charged to the issuing engine's queue; one queue serializes ~100us).
"""

import numpy as np
import ml_dtypes

B, T, D, DF, S = 4, 2048, 1024, 2048, 128
TQ = T // 2  # tokens per core (query half == owned kv half)
N_CORES = 8
BF16 = ml_dtypes.bfloat16
PAIRS = [[0, 1], [2, 3], [4, 5], [6, 7]]

_NC = {}


def _build_nc(with_vbias=True, with_obias=True):
    import concourse.mybir as mybir
    import concourse.tile as tile
    from concourse import bacc
    from concourse.bass import ts, ds

    f32 = mybir.dt.float32
    bf16 = mybir.dt.bfloat16
    AF = mybir.ActivationFunctionType
    OP = mybir.AluOpType

    nc = bacc.Bacc("TRN2", dynamic_dma_scratch_size=4096)

    # ---- I/O ----
    xt_d = nc.dram_tensor("xt", [128, 8, TQ], bf16, kind="ExternalInput")
    wu_d = nc.dram_tensor("wu", [128, 8, DF], bf16, kind="ExternalInput")
    wv_d = nc.dram_tensor("wv", [128, 8, DF], bf16, kind="ExternalInput")
    wo_d = nc.dram_tensor("wo", [128, 16, D], bf16, kind="ExternalInput")
    wqk_d = nc.dram_tensor("wqk", [128, 8, S], bf16, kind="ExternalInput")
    bu_d = nc.dram_tensor("bu", [128, 16], f32, kind="ExternalInput")
    bqk_d = nc.dram_tensor("bqk", [128, 1], f32, kind="ExternalInput")
    bv_d = nc.dram_tensor("bv", [1, DF], bf16, kind="ExternalInput") if with_vbias else None
    boe_d = nc.dram_tensor("boe", [128, D], f32, kind="ExternalInput") if with_obias else None
    qkg_d = nc.dram_tensor("qkg", [128, 4], f32, kind="ExternalInput")
    pairm_d = nc.dram_tensor("pairm", [128, 2], f32, kind="ExternalInput")
    mask_d = nc.dram_tensor("mask", [128, 16], f32, kind="ExternalInput")
    ones_r_d = nc.dram_tensor("ones_r", [1, 128], bf16, kind="ExternalInput") if with_vbias else None
    ones_c_d = nc.dram_tensor("ones_c", [128, 1], bf16, kind="ExternalInput")
    y_d = nc.dram_tensor("y", [TQ, D], f32, kind="ExternalOutput")

    lean = not (with_vbias or with_obias)

    with tile.TileContext(nc) as tc:
        with (
            tc.tile_pool(name="res", bufs=1) as res,
            tc.tile_pool(name="bigw", bufs=1) as bigw,
            tc.tile_pool(name="ps", bufs=5, space="PSUM") as psp,
            tc.tile_pool(name="pssc", bufs=2, space="PSUM") as pssc,
            tc.tile_pool(name="psden", bufs=1, space="PSUM") as psden,
            tc.tile_pool(name="dram", bufs=1, space="DRAM") as dram,
        ):
            # ---- resident tiles ----
            v_sb = res.tile([128, 16, DF], bf16)      # [tk%128, tk//128, f] rotated
            uT_sb = res.tile([128, 16, TQ], bf16)     # [f%128, f//128, tq]
            qT_sb = res.tile([128, TQ], bf16)         # [s, tq]
            kT_sb = res.tile([128, T], bf16)          # [s, tk] rotated (own first)
            bu_sb = res.tile([128, 16], f32)
            bqk_sb = res.tile([128, 1], f32)
            bv_sb = res.tile([1, DF], bf16, name="bv_sb") if with_vbias else None
            boe_sb = res.tile([128, D], f32, name="boe_sb") if with_obias else None
            qkg_sb = res.tile([128, 4], f32)
            pairm_sb = res.tile([128, 2], f32)
            mask_sb = res.tile([128, 16], f32)
            ones_r = res.tile([1, 128], bf16, name="ones_r") if with_vbias else None
            ones_c = res.tile([128, 1], bf16)
            inv_sb = res.tile([128, 8], f32)          # 1/den per tq 128-slice

            # wv, then wo in the same slot (Tile waits for the v matmuls)
            wv_sb = bigw.tile([128, 8, DF], bf16, tag="bigw")

            # DRAM bounce buffers for the sum exchange.  in = [shard0;shard1]
            # both holding own data (p-major, matching the SBUF image); out =
            # own + partner.
            cck_in = dram.tile([2, 128, TQ], bf16)
            cck_out = dram.tile([128, TQ], bf16)
            ccv0_in = dram.tile([2, 128, 8 * TQ], bf16)
            ccv0_out = dram.tile([128, 8 * TQ], bf16)
            ccv1_in = dram.tile([2, 128, 4 * TQ], bf16)
            ccv1_out = dram.tile([128, 4 * TQ], bf16)
            ccv2_in = dram.tile([2, 128, 4 * TQ], bf16)
            ccv2_out = dram.tile([128, 4 * TQ], bf16)

            with (
                tc.tile_pool(name="proj", bufs=1) as proj,
                tc.tile_pool(name="stg", bufs=8) as stg,
            ):
                xt_sb = proj.tile([128, 8, TQ], bf16)
                wqk_sb = proj.tile([128, 8, S], bf16)
                z_sb = proj.tile([128, TQ], f32)
                wu_sb = proj.tile([128, 8, DF], bf16)
                # DMA issue order = need order; xt/wv/wu split per d-plane so
                # the first matmuls start after ~1 plane instead of MBs
                nc.sync.dma_start(wqk_sb[:], wqk_d[:])
                nc.sync.dma_start(bqk_sb[:], bqk_d[:])
                for kd in range(8):
                    nc.sync.dma_start(xt_sb[:, kd, :], xt_d[:, kd, :])
                nc.sync.dma_start(qkg_sb[:], qkg_d[:])
                nc.sync.dma_start(pairm_sb[:], pairm_d[:])
                if with_vbias:
                    nc.sync.dma_start(bv_sb[:], bv_d[:])
                    nc.sync.dma_start(ones_r[:], ones_r_d[:])
                nc.sync.dma_start(bu_sb[:], bu_d[:])
                for kd in range(8):
                    nc.scalar.dma_start(wv_sb[:, kd, :], wv_d[:, kd, :])
                nc.sync.dma_start(mask_sb[:], mask_d[:])
                nc.sync.dma_start(ones_c[:], ones_c_d[:])
                if with_obias:
                    nc.sync.dma_start(boe_sb[:], boe_d[:])

                # ---- z = silu(Wqk^T xt^T + bqk) (own tokens); qT, k_own ----
                for tc_i in range(2):
                    ps = psp.tile([128, 512], f32, tag="ps")
                    for kd in range(8):
                        nc.tensor.matmul(
                            ps[:],
                            wqk_sb[:, kd, :],
                            xt_sb[:, kd, ts(tc_i, 512)],
                            start=(kd == 0),
                            stop=(kd == 7),
                        )
                    nc.scalar.activation(
                        z_sb[:, ts(tc_i, 512)], ps[:], AF.Silu, bias=bqk_sb[:, 0:1]
                    )
                nc.vector.tensor_scalar(
                    qT_sb[:], z_sb[:], qkg_sb[:, 0:1], qkg_sb[:, 1:2], OP.mult, OP.add
                )
                nc.vector.tensor_scalar(
                    kT_sb[:, 0:TQ], z_sb[:], qkg_sb[:, 2:3], qkg_sb[:, 3:4],
                    OP.mult, OP.add,
                )
                # k exchange (tiny, launches ~10us in): shard s gets k*m_s
                for s_i in range(2):
                    kstg = stg.tile([128, TQ], bf16, tag="stg", name=f"kstg{s_i}")
                    nc.vector.tensor_scalar(
                        kstg[:], kT_sb[:, 0:TQ],
                        pairm_sb[:, s_i : s_i + 1], None, OP.mult,
                    )
                    nc.gpsimd.dma_start(cck_in[s_i, :, :], kstg[:])
                nc.gpsimd.collective_compute(
                    "ReduceScatter", OP.add, replica_groups=PAIRS,
                    ins=[cck_in[:]], outs=[cck_out[:]],
                )

                # ---- v = silu(x Wv + bv), own tokens -> v_sb j tiles 0..7.
                # Exchange chunks (f 0:1024, 1024:1536, 1536:2048) each
                # launch as soon as their slice is staged, so the last RS
                # lands before the attnv ftg that consumes it ----
                def v_chunk(j, fcol):
                    ps = psp.tile([128, 512], f32, tag="ps", name="vps")
                    for kd in range(8):
                        nc.tensor.matmul(
                            ps[:],
                            xt_sb[:, kd, ts(j, 128)],
                            wv_sb[:, kd, ds(fcol, 512)],
                            start=(kd == 0),
                            stop=(kd == 7 and not with_vbias),
                        )
                    if with_vbias:
                        nc.tensor.matmul(
                            ps[:],
                            ones_r[0:1, :],
                            bv_sb[0:1, ds(fcol, 512)],
                            start=False,
                            stop=True,
                        )
                    nc.scalar.activation(v_sb[:, j, ds(fcol, 512)], ps[:], AF.Silu)

                def v_rs(cin, cout):
                    nc.gpsimd.collective_compute(
                        "ReduceScatter", OP.add, replica_groups=PAIRS,
                        ins=[cin[:]], outs=[cout[:]],
                    )

                def v_stage(j, fcol, width, ccv_in):
                    for s_i in range(2):
                        vstg = stg.tile(
                            [128, TQ], bf16, tag="stg", name=f"vstg{s_i}"
                        )
                        nc.vector.tensor_scalar(
                            vstg[:, 0:width], v_sb[:, j, ds(fcol, width)],
                            pairm_sb[:, s_i : s_i + 1], None, OP.mult,
                        )
                        nc.sync.dma_start(
                            ccv_in[s_i, :, ds(j * width, width)],
                            vstg[:, 0:width],
                        )

                for j in range(8):  # f-half 0, j-major
                    for fc in range(2):
                        v_chunk(j, fc * 512)
                    v_stage(j, 0, 1024, ccv0_in)
                v_rs(ccv0_in, ccv0_out)
                # k readback: the RS output IS the partner half.  All
                # readbacks ride the gpsimd SWDGE queue, where they sit
                # after the collectives that gate them -- on the hw DMA
                # queues their late completion would poison the cumulative
                # queue counts later barriers wait on.
                nc.sync.dma_start(kT_sb[:, TQ:T], cck_out[:])
                # wu lands here on the sync queue: after the f-half-0
                # staging (which gates RS-c1) but before u needs it
                for kd in range(8):
                    nc.sync.dma_start(wu_sb[:, kd, :], wu_d[:, kd, :])
                for fc in range(2):  # f-half 1, fc-major 512-col chunks
                    ccv_in = ccv1_in if fc == 0 else ccv2_in
                    fcol = 1024 + fc * 512
                    for j in range(8):
                        v_chunk(j, fcol)
                        v_stage(j, fcol, 512, ccv_in)
                    v_rs(ccv_in, ccv1_out if fc == 0 else ccv2_out)

                # ---- uT = silu(Wu^T xt^T + bu)  [f, tq] ----
                for ft in range(16):
                    for qc in range(2):
                        ps = psp.tile([128, 512], f32, tag="ps")
                        for kd in range(8):
                            nc.tensor.matmul(
                                ps[:],
                                wu_sb[:, kd, ts(ft, 128)],
                                xt_sb[:, kd, ts(qc, 512)],
                                start=(kd == 0),
                                stop=(kd == 7),
                            )
                        nc.scalar.activation(
                            uT_sb[:, ft, ts(qc, 512)],
                            ps[:],
                            AF.Silu,
                            bias=bu_sb[:, ft : ft + 1],
                        )

            # v f-half-0 readback into j tiles 8..15
            nc.sync.dma_start(v_sb[:, 8:16, 0:1024], ccv0_out[:])

            with (
                tc.tile_pool(name="attn", bufs=(4 if lean else 3)) as attn,
                tc.tile_pool(name="yout", bufs=2) as yout,
            ):
                wo_sb = bigw.tile([128, 16, D], bf16, tag="bigw")
                pT = [
                    attn.tile([128, 16, 512], bf16, tag="at", name=f"pT{q}")
                    for q in range(2)
                ]
                oT = [
                    attn.tile([128, 16, 512], bf16, tag="at", name=f"oT{q}")
                    for q in range(2)
                ]

                def scores(qc):
                    for j in range(16):
                        ps = pssc.tile([128, 512], f32, tag="pssc")
                        nc.tensor.matmul(
                            ps[:],
                            kT_sb[:, ts(j, 128)],
                            qT_sb[:, ts(qc, 512)],
                            start=True,
                            stop=True,
                        )
                        nc.scalar.activation(
                            pT[qc][:, j, :], ps[:], AF.Exp,
                            bias=mask_sb[:, j : j + 1],
                        )

                def attnv(qc, ftg):
                    # oT = (v^T pT) * uT; 4 parallel psum banks so each j's
                    # 4 matmuls start as soon as exp_j lands
                    pss = [
                        psp.tile([128, 512], f32, tag="ps", name=f"ot_ps{i}")
                        for i in range(4)
                    ]
                    for j in range(16):
                        for i in range(4):
                            ft = ftg * 4 + i
                            nc.tensor.matmul(
                                pss[i][:],
                                v_sb[:, j, ts(ft, 128)],
                                pT[qc][:, j, :],
                                start=(j == 0),
                                stop=(j == 15),
                            )
                    for i in range(4):
                        ft = ftg * 4 + i
                        nc.vector.tensor_mul(
                            oT[qc][:, ft, :], pss[i][:], uT_sb[:, ft, ts(qc, 512)]
                        )

                def dens(qc):
                    for sl in range(4):
                        dps = psden.tile([128, 1], f32, tag="den")
                        for j in range(16):
                            nc.tensor.matmul(
                                dps[:],
                                pT[qc][:, j, ts(sl, 128)],
                                ones_c[:, 0:1],
                                start=(j == 0),
                                stop=(j == 15),
                            )
                        nc.vector.reciprocal(
                            inv_sb[:, qc * 4 + sl : qc * 4 + sl + 1], dps[:]
                        )

                def yout_qc(qc):
                    for sl in range(4):
                        y_sb = yout.tile([128, D], f32, tag="y")
                        for dc in range(2):
                            ps = psp.tile([128, 512], f32, tag="ps")
                            for ft in range(16):
                                nc.tensor.matmul(
                                    ps[:],
                                    oT[qc][:, ft, ts(sl, 128)],
                                    wo_sb[:, ft, ts(dc, 512)],
                                    start=(ft == 0),
                                    stop=(ft == 15),
                                )
                            nc.vector.tensor_scalar(
                                y_sb[:, ts(dc, 512)],
                                ps[:],
                                inv_sb[:, qc * 4 + sl : qc * 4 + sl + 1],
                                None,
                                OP.mult,
                            )
                        if with_obias:
                            nc.vector.tensor_add(y_sb[:], y_sb[:], boe_sb[:])
                        nc.sync.dma_start(
                            y_d[ds(qc * 512 + sl * 128, 128), :], y_sb[:]
                        )

                # f-half-0 attention for both query chunks first; the
                # f-half-1 exchanges land meanwhile.  dens sit after an
                # attnv group so the PE queue never waits on the exp tail.
                # Readbacks/subtracts are emitted just before their first
                # consumer so the DVE queue never blocks on a late RS.
                scores(0)
                # wo reuses the wv slot.  The tiny copies below (sourced from
                # pT0, which only exists after the Silu->Exp act-table switch)
                # pin each wo DMA chunk behind the table-switch barrier via
                # WAW -- otherwise those DMAs land before the barrier and
                # their (collective-delayed) queue slots gate the exps.
                # wo itself is not needed until the y matmuls, much later.
                for fg in range(4):
                    nc.scalar.activation(
                        wo_sb[0:1, 4 * fg, 0:1], pT[0][0:1, 0, 0:1], AF.Copy
                    )
                for fg in range(4):
                    nc.sync.dma_start(
                        wo_sb[:, 4 * fg : 4 * fg + 4, :],
                        wo_d[:, 4 * fg : 4 * fg + 4, :],
                    )

                attnv(0, 0)
                dens(0)
                scores(1)
                attnv(0, 1)
                attnv(1, 0)
                dens(1)
                attnv(1, 1)
                nc.sync.dma_start(v_sb[:, 8:16, 1024:1536], ccv1_out[:])
                attnv(0, 2)
                attnv(1, 2)
                nc.sync.dma_start(v_sb[:, 8:16, 1536:2048], ccv2_out[:])
                attnv(0, 3)
                attnv(1, 3)
                yout_qc(0)
                yout_qc(1)

    nc.compile()
    return nc


def _get_nc(with_vbias=True, with_obias=True):
    key = (with_vbias, with_obias)
    if key not in _NC:
        _NC[key] = _build_nc(*key)
    return _NC[key]


def _prep_in_maps(inputs, with_vbias=True, with_obias=True):
    x = np.ascontiguousarray(inputs["x"], dtype=np.float32)
    length = np.asarray(inputs["length"]).astype(np.int64)
    Wu = np.asarray(inputs["Wu_w"], np.float32)
    bu = np.asarray(inputs["Wu_b"], np.float32)
    Wv = np.asarray(inputs["Wv_w"], np.float32)
    bv = np.asarray(inputs["Wv_b"], np.float32)
    Wqk = np.asarray(inputs["Wqk_w"], np.float32)
    bqk = np.asarray(inputs["Wqk_b"], np.float32)
    Wo = np.asarray(inputs["Wo_w"], np.float32)
    bo = np.asarray(inputs["Wo_b"], np.float32)
    gamma = np.asarray(inputs["gamma"], np.float32)
    beta = np.asarray(inputs["beta"], np.float32)
    u_qk = np.asarray(inputs["u_qk"], np.float32)

    inv_s = np.float32(1.0 / np.sqrt(S))
    qkg = np.stack(
        [gamma[0] * inv_s, beta[0] * inv_s + u_qk, gamma[1], beta[1]], axis=1
    ).astype(np.float32)  # [128, 4]

    def pack_w(w, ko):  # [K, N] -> [128, ko, N] (k = o*128 + p)
        return np.ascontiguousarray(
            w.reshape(ko, 128, w.shape[1]).transpose(1, 0, 2).astype(BF16)
        )

    wu_p = pack_w(Wu, 8)
    wv_p = pack_w(Wv, 8)
    wo_p = pack_w(Wo, 16)
    wqk_p = pack_w(Wqk, 8)
    bu_p = np.ascontiguousarray(bu.reshape(16, 128).T.astype(np.float32))
    bqk_p = np.ascontiguousarray(bqk[:, None].astype(np.float32))
    bv_p = np.ascontiguousarray(bv[None, :].astype(BF16))
    boe_p = np.ascontiguousarray(np.broadcast_to(bo[None, :], (128, D)).astype(np.float32))
    ones_r = np.ones((1, 128), BF16)
    ones_c = np.ones((128, 1), BF16)

    in_maps = []
    for c in range(N_CORES):
        b, h = c // 2, c % 2
        # own token half only
        xb = x[b, h * TQ:(h + 1) * TQ]  # [TQ, D]
        xT = xb.T.astype(BF16)  # [D, TQ]
        xT_p = np.ascontiguousarray(xT.reshape(8, 128, TQ).transpose(1, 0, 2))
        # mask follows the rotated key order (own half first)
        valid = np.arange(T) < int(length[b])
        vrot = np.concatenate(
            [valid[h * TQ:(h + 1) * TQ], valid[(1 - h) * TQ:(2 - h) * TQ]]
        )
        mask = np.where(vrot, np.float32(0.0), np.float32(-1e30))
        mask_p = np.ascontiguousarray(mask.reshape(16, 128).T.astype(np.float32))
        # shard masks: my data goes only into the partner's RS shard
        pairm = np.ascontiguousarray(np.broadcast_to(
            np.array([0.0 if s == h else 1.0 for s in range(2)], np.float32),
            (128, 2),
        ))
        m = {
            "xt": xT_p,
            "wu": wu_p,
            "wv": wv_p,
            "wo": wo_p,
            "wqk": wqk_p,
            "bu": bu_p,
            "bqk": bqk_p,
            "qkg": qkg,
            "pairm": pairm,
            "mask": mask_p,
            "ones_c": ones_c,
        }
        if with_vbias:
            m["bv"] = bv_p
            m["ones_r"] = ones_r
        if with_obias:
            m["boe"] = boe_p
        in_maps.append(m)
    return in_maps


def _gather(results):
    y = np.empty((B, T, D), np.float32)
    for c in range(N_CORES):
        b, h = c // 2, c % 2
        y[b, h * TQ:(h + 1) * TQ, :] = results[c]["y"]
    return y


def _variant(inputs):
    with_vbias = bool(np.any(np.asarray(inputs["Wv_b"])))
    with_obias = bool(np.any(np.asarray(inputs["Wo_b"])))
    return with_vbias, with_obias


def _run(inputs, trace=False):
    from concourse.bass_utils import run_bass_kernel_spmd

    wv, wo = _variant(inputs)
    nc = _get_nc(wv, wo)
    in_maps = _prep_in_maps(inputs, wv, wo)
    res = run_bass_kernel_spmd(
        nc, in_maps, core_ids=list(range(N_CORES)), trace=trace
    )
    return _gather(res.results), res


def kernel(**inputs) -> np.ndarray:
    out, _ = _run(inputs)
    return out


# revision 51
# speedup vs baseline: 1.0374x; 1.0374x over previous
"""GAU (gated attention unit) Trainium2 Bass kernel, 8-core SPMD.

Problem: B=4, T=2048, D=1024, DF=2048, S=128, fp32 in/out.
  u = silu(x@Wu+bu); v = silu(x@Wv+bv); z = silu(x@Wqk+bqk)
  q = (z*g0+b0)/sqrt(S); k = z*g1+b1
  scores = (q+u_qk) @ k^T, key-masked by length[b]; attn = softmax
  out = u * (attn@v); y = out@Wo + bo

Sharding: core c -> batch b=c//2, token half h=c%2 (1024 tokens).
Each core computes u/q for its own token half (its query half) AND
v/k only for its own token half; the partner's k/v half arrives via a
pair-wise "sum exchange":

  Each core writes (k, v) * m_s into shard s of a ReduceScatter(add)
  input, where the host supplies m = (0,1) on even cores and (1,0) on
  odd cores.  Rank r's RS output is shard r = 0 + partner's data, so
  every core receives exactly its partner's (k, v) -- SPMD-symmetric
  (no core-dependent addressing), numerically exact, and the
  collective is priced/sized on the RECEIVED bytes only.

Three RSs pipeline under the projections: k (tiny, early), then
v[:, :1024], then v[:, 1024:], each launched as soon as its slice is
staged.  This removes the duplicated full-T v/z compute of the pure
data-parallel layout (~46 us of PE time per core).

Keys are in LOCAL-ROTATED order (own half first), as in the mask the
host builds; attention is key-permutation invariant.

Layout strategy (everything stays transposed so no on-device
transposes are needed; host pre-transposes x and pre-packs weights):
  xtT  [d, tq]  : rhs/lhsT for all projections (contract d on partitions)
  z    [s, tq]  = (Wqk lhsT) @ (xtT rhs)         -> qT [s,tq], k_own
  v    [tk, f]  = (xtT lhsT) @ (Wv rhs)          own j tiles 0..7
  uT   [f, tq]  = (Wu lhsT) @ (xtT rhs)
  pT   [tk,tq]  = exp((kT lhsT)@(qT rhs) + mask) softmax numerator
  den  [tq, 1]  = (pT lhsT) @ (ones rhs)         per-query denominator
  oT   [f, tq]  = (v lhsT) @ (pT rhs), gated *uT
  y    [tq, d]  = (oT lhsT) @ (Wo rhs), *1/den, +bo

All matmuls in bf16 with fp32 PSUM accumulation. Softmax skips the
row-max subtraction: pre-softmax logits for this operator are
|s| <~ 2 (q is scaled by gamma*0.02-ish weights and 1/sqrt(S)), so
exp() cannot overflow; masked keys get a -1e30 bias -> exp==0 exactly.

DMAs are spread across the sync/scalar/gpsimd queues (a DMA's cost is
charged to the issuing engine's queue; one queue serializes ~100us).
"""

import numpy as np
import ml_dtypes

B, T, D, DF, S = 4, 2048, 1024, 2048, 128
TQ = T // 2  # tokens per core (query half == owned kv half)
N_CORES = 8
BF16 = ml_dtypes.bfloat16
PAIRS = [[0, 1], [2, 3], [4, 5], [6, 7]]

_NC = {}


def _build_nc(with_vbias=True, with_obias=True):
    import concourse.mybir as mybir
    import concourse.tile as tile
    from concourse import bacc
    from concourse.bass import ts, ds

    f32 = mybir.dt.float32
    bf16 = mybir.dt.bfloat16
    AF = mybir.ActivationFunctionType
    OP = mybir.AluOpType

    nc = bacc.Bacc("TRN2", dynamic_dma_scratch_size=4096)

    # ---- I/O ----
    xt_d = nc.dram_tensor("xt", [128, 8, TQ], bf16, kind="ExternalInput")
    wu_d = nc.dram_tensor("wu", [128, 8, DF], bf16, kind="ExternalInput")
    wv_d = nc.dram_tensor("wv", [128, 8, DF], bf16, kind="ExternalInput")
    wo_d = nc.dram_tensor("wo", [128, 16, D], bf16, kind="ExternalInput")
    wqk_d = nc.dram_tensor("wqk", [128, 8, S], bf16, kind="ExternalInput")
    bu_d = nc.dram_tensor("bu", [128, 16], f32, kind="ExternalInput")
    bqk_d = nc.dram_tensor("bqk", [128, 1], f32, kind="ExternalInput")
    bv_d = nc.dram_tensor("bv", [1, DF], bf16, kind="ExternalInput") if with_vbias else None
    boe_d = nc.dram_tensor("boe", [128, D], f32, kind="ExternalInput") if with_obias else None
    qkg_d = nc.dram_tensor("qkg", [128, 4], f32, kind="ExternalInput")
    pairm_d = nc.dram_tensor("pairm", [128, 2], f32, kind="ExternalInput")
    mask_d = nc.dram_tensor("mask", [128, 16], f32, kind="ExternalInput")
    ones_r_d = nc.dram_tensor("ones_r", [1, 128], bf16, kind="ExternalInput") if with_vbias else None
    ones_c_d = nc.dram_tensor("ones_c", [128, 1], bf16, kind="ExternalInput")
    y_d = nc.dram_tensor("y", [TQ, D], f32, kind="ExternalOutput")

    lean = not (with_vbias or with_obias)

    with tile.TileContext(nc) as tc:
        with (
            tc.tile_pool(name="res", bufs=1) as res,
            tc.tile_pool(name="bigw", bufs=1) as bigw,
            tc.tile_pool(name="ps", bufs=5, space="PSUM") as psp,
            tc.tile_pool(name="pssc", bufs=2, space="PSUM") as pssc,
            tc.tile_pool(name="psden", bufs=1, space="PSUM") as psden,
            tc.tile_pool(name="dram", bufs=1, space="DRAM") as dram,
        ):
            # ---- resident tiles ----
            v_sb = res.tile([128, 16, DF], bf16)      # [tk%128, tk//128, f] rotated
            uT_sb = res.tile([128, 16, TQ], bf16)     # [f%128, f//128, tq]
            qT_sb = res.tile([128, TQ], bf16)         # [s, tq]
            kT_sb = res.tile([128, T], bf16)          # [s, tk] rotated (own first)
            bu_sb = res.tile([128, 16], f32)
            bqk_sb = res.tile([128, 1], f32)
            bv_sb = res.tile([1, DF], bf16, name="bv_sb") if with_vbias else None
            boe_sb = res.tile([128, D], f32, name="boe_sb") if with_obias else None
            qkg_sb = res.tile([128, 4], f32)
            pairm_sb = res.tile([128, 2], f32)
            mask_sb = res.tile([128, 16], f32)
            ones_r = res.tile([1, 128], bf16, name="ones_r") if with_vbias else None
            ones_c = res.tile([128, 1], bf16)
            inv_sb = res.tile([128, 8], f32)          # 1/den per tq 128-slice

            # wv, then wo in the same slot (Tile waits for the v matmuls)
            wv_sb = bigw.tile([128, 8, DF], bf16, tag="bigw")

            # DRAM bounce buffers for the sum exchange.  in = [shard0;shard1]
            # both holding own data (p-major, matching the SBUF image); out =
            # own + partner.
            cck_in = dram.tile([2, 128, TQ], bf16)
            cck_out = dram.tile([128, TQ], bf16)
            ccv0_in = dram.tile([2, 128, 8 * TQ], bf16)
            ccv0_out = dram.tile([128, 8 * TQ], bf16)
            ccv1_in = dram.tile([2, 128, 4 * TQ], bf16)
            ccv1_out = dram.tile([128, 4 * TQ], bf16)
            ccv2_in = dram.tile([2, 128, 4 * TQ], bf16)
            ccv2_out = dram.tile([128, 4 * TQ], bf16)

            with (
                tc.tile_pool(name="proj", bufs=1) as proj,
                tc.tile_pool(name="stg", bufs=4) as stg,
                tc.tile_pool(name="stgb", bufs=8) as stgb,
            ):
                xt_sb = proj.tile([128, 8, TQ], bf16)
                wqk_sb = proj.tile([128, 8, S], bf16)
                z_sb = proj.tile([128, TQ], f32)
                wu_sb = proj.tile([128, 8, DF], bf16)
                # DMA issue order = need order; xt/wv/wu split per d-plane so
                # the first matmuls start after ~1 plane instead of MBs
                nc.sync.dma_start(wqk_sb[:], wqk_d[:])
                nc.sync.dma_start(bqk_sb[:], bqk_d[:])
                for kd in range(8):
                    nc.sync.dma_start(xt_sb[:, kd, :], xt_d[:, kd, :])
                nc.sync.dma_start(qkg_sb[:], qkg_d[:])
                nc.sync.dma_start(pairm_sb[:], pairm_d[:])
                if with_vbias:
                    nc.sync.dma_start(bv_sb[:], bv_d[:])
                    nc.sync.dma_start(ones_r[:], ones_r_d[:])
                nc.sync.dma_start(bu_sb[:], bu_d[:])
                for kd in range(8):
                    # split across the (idle) gpsimd and scalar queues so all
                    # planes land by ~10us and the first v chunk isn't gated
                    # on a 13us serial weight load
                    eng = nc.gpsimd if kd < 4 else nc.scalar
                    eng.dma_start(wv_sb[:, kd, :], wv_d[:, kd, :])
                nc.sync.dma_start(mask_sb[:], mask_d[:])
                nc.sync.dma_start(ones_c[:], ones_c_d[:])
                if with_obias:
                    nc.sync.dma_start(boe_sb[:], boe_d[:])

                # ---- z = silu(Wqk^T xt^T + bqk) (own tokens); qT, k_own ----
                for tc_i in range(2):
                    ps = psp.tile([128, 512], f32, tag="ps")
                    for kd in range(8):
                        nc.tensor.matmul(
                            ps[:],
                            wqk_sb[:, kd, :],
                            xt_sb[:, kd, ts(tc_i, 512)],
                            start=(kd == 0),
                            stop=(kd == 7),
                        )
                    nc.scalar.activation(
                        z_sb[:, ts(tc_i, 512)], ps[:], AF.Silu, bias=bqk_sb[:, 0:1]
                    )
                nc.vector.tensor_scalar(
                    qT_sb[:], z_sb[:], qkg_sb[:, 0:1], qkg_sb[:, 1:2], OP.mult, OP.add
                )
                nc.vector.tensor_scalar(
                    kT_sb[:, 0:TQ], z_sb[:], qkg_sb[:, 2:3], qkg_sb[:, 3:4],
                    OP.mult, OP.add,
                )
                # k exchange (tiny, launches ~10us in): shard s gets k*m_s
                for s_i in range(2):
                    kstg = stg.tile([128, TQ], bf16, tag="stg", name=f"kstg{s_i}")
                    nc.vector.tensor_scalar(
                        kstg[:], kT_sb[:, 0:TQ],
                        pairm_sb[:, s_i : s_i + 1], None, OP.mult,
                    )
                    nc.gpsimd.dma_start(cck_in[s_i, :, :], kstg[:])
                nc.gpsimd.collective_compute(
                    "ReduceScatter", OP.add, replica_groups=PAIRS,
                    ins=[cck_in[:]], outs=[cck_out[:]],
                )

                # ---- v = silu(x Wv + bv), own tokens -> v_sb j tiles 0..7.
                # Exchange chunks (f 0:1024, 1024:1536, 1536:2048) each
                # launch as soon as their slice is staged, so the last RS
                # lands before the attnv ftg that consumes it ----
                def v_chunk(j, fcol):
                    ps = psp.tile([128, 512], f32, tag="ps", name="vps")
                    for kd in range(8):
                        nc.tensor.matmul(
                            ps[:],
                            xt_sb[:, kd, ts(j, 128)],
                            wv_sb[:, kd, ds(fcol, 512)],
                            start=(kd == 0),
                            stop=(kd == 7 and not with_vbias),
                        )
                    if with_vbias:
                        nc.tensor.matmul(
                            ps[:],
                            ones_r[0:1, :],
                            bv_sb[0:1, ds(fcol, 512)],
                            start=False,
                            stop=True,
                        )
                    nc.scalar.activation(v_sb[:, j, ds(fcol, 512)], ps[:], AF.Silu)

                def v_rs(cin, cout):
                    nc.gpsimd.collective_compute(
                        "ReduceScatter", OP.add, replica_groups=PAIRS,
                        ins=[cin[:]], outs=[cout[:]],
                    )

                def v_stage(j, fcol, width, ccv_in):
                    pool = stg if width == 1024 else stgb
                    for s_i in range(2):
                        vstg = pool.tile(
                            [128, width], bf16, tag="stg", name=f"vstg{s_i}"
                        )
                        nc.vector.tensor_scalar(
                            vstg[:, 0:width], v_sb[:, j, ds(fcol, width)],
                            pairm_sb[:, s_i : s_i + 1], None, OP.mult,
                        )
                        nc.sync.dma_start(
                            ccv_in[s_i, :, ds(j * width, width)],
                            vstg[:, 0:width],
                        )

                for j in range(8):  # f-half 0, j-major
                    for fc in range(2):
                        v_chunk(j, fc * 512)
                    v_stage(j, 0, 1024, ccv0_in)
                v_rs(ccv0_in, ccv0_out)
                # k readback: the RS output IS the partner half
                nc.sync.dma_start(kT_sb[:, TQ:T], cck_out[:])
                # wu lands here on the sync queue: after the f-half-0
                # staging (which gates RS-c1) but before u needs it
                for kd in range(8):
                    nc.sync.dma_start(wu_sb[:, kd, :], wu_d[:, kd, :])
                for fc in range(2):  # f-half 1, fc-major 512-col chunks
                    ccv_in = ccv1_in if fc == 0 else ccv2_in
                    fcol = 1024 + fc * 512
                    for j in range(8):
                        v_chunk(j, fcol)
                        v_stage(j, fcol, 512, ccv_in)
                    v_rs(ccv_in, ccv1_out if fc == 0 else ccv2_out)

                # ---- uT = silu(Wu^T xt^T + bu)  [f, tq] ----
                for ft in range(16):
                    for qc in range(2):
                        ps = psp.tile([128, 512], f32, tag="ps")
                        for kd in range(8):
                            nc.tensor.matmul(
                                ps[:],
                                wu_sb[:, kd, ts(ft, 128)],
                                xt_sb[:, kd, ts(qc, 512)],
                                start=(kd == 0),
                                stop=(kd == 7),
                            )
                        nc.scalar.activation(
                            uT_sb[:, ft, ts(qc, 512)],
                            ps[:],
                            AF.Silu,
                            bias=bu_sb[:, ft : ft + 1],
                        )

            with (
                tc.tile_pool(name="attn", bufs=(4 if lean else 3)) as attn,
                tc.tile_pool(name="yout", bufs=2) as yout,
            ):
                wo_sb = bigw.tile([128, 16, D], bf16, tag="bigw")
                pT = [
                    attn.tile([128, 16, 512], bf16, tag="at", name=f"pT{q}")
                    for q in range(2)
                ]
                oT = [
                    attn.tile([128, 16, 512], bf16, tag="at", name=f"oT{q}")
                    for q in range(2)
                ]

                def scores(qc):
                    for j in range(16):
                        ps = pssc.tile([128, 512], f32, tag="pssc")
                        nc.tensor.matmul(
                            ps[:],
                            kT_sb[:, ts(j, 128)],
                            qT_sb[:, ts(qc, 512)],
                            start=True,
                            stop=True,
                        )
                        nc.scalar.activation(
                            pT[qc][:, j, :], ps[:], AF.Exp,
                            bias=mask_sb[:, j : j + 1],
                        )

                def attnv(qc, ftg):
                    # oT = (v^T pT) * uT; 4 parallel psum banks so each j's
                    # 4 matmuls start as soon as exp_j lands
                    pss = [
                        psp.tile([128, 512], f32, tag="ps", name=f"ot_ps{i}")
                        for i in range(4)
                    ]
                    for j in range(16):
                        for i in range(4):
                            ft = ftg * 4 + i
                            nc.tensor.matmul(
                                pss[i][:],
                                v_sb[:, j, ts(ft, 128)],
                                pT[qc][:, j, :],
                                start=(j == 0),
                                stop=(j == 15),
                            )
                    for i in range(4):
                        ft = ftg * 4 + i
                        nc.vector.tensor_mul(
                            oT[qc][:, ft, :], pss[i][:], uT_sb[:, ft, ts(qc, 512)]
                        )

                def dens(qc):
                    for sl in range(4):
                        dps = psden.tile([128, 1], f32, tag="den")
                        for j in range(16):
                            nc.tensor.matmul(
                                dps[:],
                                pT[qc][:, j, ts(sl, 128)],
                                ones_c[:, 0:1],
                                start=(j == 0),
                                stop=(j == 15),
                            )
                        nc.vector.reciprocal(
                            inv_sb[:, qc * 4 + sl : qc * 4 + sl + 1], dps[:]
                        )

                def yout_qc(qc):
                    for sl in range(4):
                        y_sb = yout.tile([128, D], f32, tag="y")
                        for dc in range(2):
                            ps = psp.tile([128, 512], f32, tag="ps")
                            for ft in range(16):
                                nc.tensor.matmul(
                                    ps[:],
                                    oT[qc][:, ft, ts(sl, 128)],
                                    wo_sb[:, ft, ts(dc, 512)],
                                    start=(ft == 0),
                                    stop=(ft == 15),
                                )
                            nc.vector.tensor_scalar(
                                y_sb[:, ts(dc, 512)],
                                ps[:],
                                inv_sb[:, qc * 4 + sl : qc * 4 + sl + 1],
                                None,
                                OP.mult,
                            )
                        if with_obias:
                            nc.vector.tensor_add(y_sb[:], y_sb[:], boe_sb[:])
                        nc.sync.dma_start(
                            y_d[ds(qc * 512 + sl * 128, 128), :], y_sb[:]
                        )

                # f-half-0 attention for both query chunks first; the
                # f-half-1 exchanges land meanwhile.  dens sit after an
                # attnv group so the PE queue never waits on the exp tail.
                # Readbacks/subtracts are emitted just before their first
                # consumer so the DVE queue never blocks on a late RS.
                scores(0)
                # wo reuses the wv slot.  The tiny copies below (sourced from
                # pT0, which only exists after the Silu->Exp act-table switch)
                # pin each wo DMA chunk behind the table-switch barrier via
                # WAW -- otherwise those DMAs land before the barrier and
                # their (collective-delayed) queue slots gate the exps.
                # wo itself is not needed until the y matmuls, much later.
                for fg in range(4):
                    nc.scalar.activation(
                        wo_sb[0:1, 4 * fg, 0:1], pT[0][0:1, 0, 0:1], AF.Copy
                    )
                for fg in range(4):
                    nc.sync.dma_start(
                        wo_sb[:, 4 * fg : 4 * fg + 4, :],
                        wo_d[:, 4 * fg : 4 * fg + 4, :],
                    )
                # v f-half-0 readback into j tiles 8..15; emitted after the
                # act-table-switch point so the pre-switch drain barrier
                # doesn't wait for it (it is gated by the RS-c1 collective)
                nc.sync.dma_start(v_sb[:, 8:16, 0:1024], ccv0_out[:])

                attnv(0, 0)
                dens(0)
                scores(1)
                attnv(0, 1)
                attnv(1, 0)
                dens(1)
                attnv(1, 1)
                nc.sync.dma_start(v_sb[:, 8:16, 1024:1536], ccv1_out[:])
                attnv(0, 2)
                attnv(1, 2)
                nc.sync.dma_start(v_sb[:, 8:16, 1536:2048], ccv2_out[:])
                attnv(0, 3)
                attnv(1, 3)
                yout_qc(0)
                yout_qc(1)

    nc.compile()
    return nc


def _get_nc(with_vbias=True, with_obias=True):
    key = (with_vbias, with_obias)
    if key not in _NC:
        _NC[key] = _build_nc(*key)
    return _NC[key]


def _prep_in_maps(inputs, with_vbias=True, with_obias=True):
    x = np.ascontiguousarray(inputs["x"], dtype=np.float32)
    length = np.asarray(inputs["length"]).astype(np.int64)
    Wu = np.asarray(inputs["Wu_w"], np.float32)
    bu = np.asarray(inputs["Wu_b"], np.float32)
    Wv = np.asarray(inputs["Wv_w"], np.float32)
    bv = np.asarray(inputs["Wv_b"], np.float32)
    Wqk = np.asarray(inputs["Wqk_w"], np.float32)
    bqk = np.asarray(inputs["Wqk_b"], np.float32)
    Wo = np.asarray(inputs["Wo_w"], np.float32)
    bo = np.asarray(inputs["Wo_b"], np.float32)
    gamma = np.asarray(inputs["gamma"], np.float32)
    beta = np.asarray(inputs["beta"], np.float32)
    u_qk = np.asarray(inputs["u_qk"], np.float32)

    inv_s = np.float32(1.0 / np.sqrt(S))
    qkg = np.stack(
        [gamma[0] * inv_s, beta[0] * inv_s + u_qk, gamma[1], beta[1]], axis=1
    ).astype(np.float32)  # [128, 4]

    def pack_w(w, ko):  # [K, N] -> [128, ko, N] (k = o*128 + p)
        return np.ascontiguousarray(
            w.reshape(ko, 128, w.shape[1]).transpose(1, 0, 2).astype(BF16)
        )

    wu_p = pack_w(Wu, 8)
    wv_p = pack_w(Wv, 8)
    wo_p = pack_w(Wo, 16)
    wqk_p = pack_w(Wqk, 8)
    bu_p = np.ascontiguousarray(bu.reshape(16, 128).T.astype(np.float32))
    bqk_p = np.ascontiguousarray(bqk[:, None].astype(np.float32))
    bv_p = np.ascontiguousarray(bv[None, :].astype(BF16))
    boe_p = np.ascontiguousarray(np.broadcast_to(bo[None, :], (128, D)).astype(np.float32))
    ones_r = np.ones((1, 128), BF16)
    ones_c = np.ones((128, 1), BF16)

    in_maps = []
    for c in range(N_CORES):
        b, h = c // 2, c % 2
        # own token half only
        xb = x[b, h * TQ:(h + 1) * TQ]  # [TQ, D]
        xT = xb.T.astype(BF16)  # [D, TQ]
        xT_p = np.ascontiguousarray(xT.reshape(8, 128, TQ).transpose(1, 0, 2))
        # mask follows the rotated key order (own half first)
        valid = np.arange(T) < int(length[b])
        vrot = np.concatenate(
            [valid[h * TQ:(h + 1) * TQ], valid[(1 - h) * TQ:(2 - h) * TQ]]
        )
        mask = np.where(vrot, np.float32(0.0), np.float32(-1e30))
        mask_p = np.ascontiguousarray(mask.reshape(16, 128).T.astype(np.float32))
        # shard masks: my data goes only into the partner's RS shard
        pairm = np.ascontiguousarray(np.broadcast_to(
            np.array([0.0 if s == h else 1.0 for s in range(2)], np.float32),
            (128, 2),
        ))
        m = {
            "xt": xT_p,
            "wu": wu_p,
            "wv": wv_p,
            "wo": wo_p,
            "wqk": wqk_p,
            "bu": bu_p,
            "bqk": bqk_p,
            "qkg": qkg,
            "pairm": pairm,
            "mask": mask_p,
            "ones_c": ones_c,
        }
        if with_vbias:
            m["bv"] = bv_p
            m["ones_r"] = ones_r
        if with_obias:
            m["boe"] = boe_p
        in_maps.append(m)
    return in_maps


def _gather(results):
    y = np.empty((B, T, D), np.float32)
    for c in range(N_CORES):
        b, h = c // 2, c % 2
        y[b, h * TQ:(h + 1) * TQ, :] = results[c]["y"]
    return y


def _variant(inputs):
    with_vbias = bool(np.any(np.asarray(inputs["Wv_b"])))
    with_obias = bool(np.any(np.asarray(inputs["Wo_b"])))
    return with_vbias, with_obias


def _run(inputs, trace=False):
    from concourse.bass_utils import run_bass_kernel_spmd

    wv, wo = _variant(inputs)
    nc = _get_nc(wv, wo)
    in_maps = _prep_in_maps(inputs, wv, wo)
    res = run_bass_kernel_spmd(
        nc, in_maps, core_ids=list(range(N_CORES)), trace=trace
    )
    return _gather(res.results), res


def kernel(**inputs) -> np.ndarray:
    out, _ = _run(inputs)
    return out
